# revision 34
# baseline (speedup 1.0000x reference)
"""GraphSAGE layer on 8 Trainium2 NeuronCores.  ~287 us HW exec
(baseline 977 us).

Strategy (1D graph partitioning):
  - Nodes (output rows / destination segments) sharded across 8 cores,
    6250 per core.  Edges are bucketed to the core owning their
    destination, sorted by (dst tile, src); the full feature table is
    replicated in DRAM on every core as bf16 rows padded to 128 cols
    (so the row stride is 256 B, required by the gather descriptor
    stride encoding).
  - Source rows are fetched with `dma_gather` (SWDGE gather, one
    descriptor per edge), round-robined over 4 SWDGE queues so all four
    Q7 core-pairs generate descriptors in parallel (~4x the single-queue
    rate; this is the critical path).  Each call is <= 1024 indices
    (the per-queue descriptor-ring cap), split evenly per (tile, half).
    Only the 128 B of real bf16 features per row are gathered (raw
    instruction builder `_dma_gather_half` skips the 256 B elem assert,
    which only the transpose path needs).  Sorting edges by src gives
    the drain monotone HBM addresses.  Indices are int16, so the table
    is split at row 32768 (A/B halves).
  - The one-hot segment matrices S ([128 edge-pos x 128 dst] * weight)
    are precomputed on the host in bf16 and streamed from DRAM per
    destination tile — no on-chip one-hot construction (the DVE
    per-partition-scalar path costs ~1.5-3 us per block).
  - Per 128-destination tile the kernel accumulates
        neighborT[64, 128] += G_block.T @ S_block
    in PSUM with single-pass bf16 matmuls (~107 ns effective each).
  - Self features arrive pre-transposed bf16 from the host; the final
    linear layer is one [128,128]x[128,64] bf16 matmul per tile, then
    bias add (DVE), sqrt(sum(x^2)+1e-24) (ACT, == max(||x||,1e-12)) and
    row scaling by 1/norm (ACT Copy with per-partition scale) in f32.
"""

import sys

if "/opt/trn_rl_repo" not in sys.path:
    sys.path.insert(0, "/opt/trn_rl_repo")

import numpy as np
import ml_dtypes

import concourse.bacc as bacc
import concourse.tile as tile
from concourse import mybir
from concourse.bass_utils import run_bass_kernel_spmd

BF16 = ml_dtypes.bfloat16


def _dma_gather_half(gp, out_ap, in_ap, idxs_ap, num_idxs, elem_size, queue_num):
    """dma_gather (non-transpose, DRAM source) with elem_size_bytes that is
    not a multiple of 256 B.  Mirrors BassGpSimd.dma_gather minus the
    256 B-elem assert — that alignment is only required by the transpose
    path; the non-transpose ucode emits one descriptor of
    elem_size_bytes per index with a row stride that must be 256 B
    aligned (stride_bytes_256 encoding)."""
    from concourse import ap_utils
    from concourse.bass import exact_div, round_up_to_multiple

    assert idxs_ap.dtype == mybir.dt.int16
    assert in_ap.dtype == out_ap.dtype
    elem_step = in_ap.ap[0][0]
    stride_bytes = elem_step * mybir.dt.size(in_ap.dtype)
    stride_bytes_256 = exact_div(stride_bytes, 256)
    assert ap_utils.ap_is_contiguous(out_ap.ap[1:])
    assert ap_utils.ap_is_contiguous(idxs_ap.ap[1:])
    assert out_ap.ap[0][1] * out_ap.ap[1][1] == round_up_to_multiple(num_idxs, 128)
    assert out_ap.ap[-1][1] == elem_size

    _in_ap = gp.lower_ap_dma(in_ap, for_custom_bir_dma=True)
    _idxs_ap = gp.lower_ap(idxs_ap)
    _out_ap = gp.lower_ap(out_ap)
    return gp.add_instruction(
        mybir.InstDMAGatherAnt(
            name=gp.bass.get_next_instruction_name(),
            ins=[
                *_in_ap,
                _idxs_ap,
                gp.lower_val_access(gp.to_reg(num_idxs)),
            ],
            outs=[_out_ap],
            transpose=False,
            num_idxs=num_idxs,
            elem_size=elem_size,
            stride_bytes_256=stride_bytes_256,
            gen_mode=0,
            single_packet=True,
            queue_num=queue_num,
            sbuf_tokens_per_rank=0,
            sbuf_free_dim_per_rank=0,
            sbuf_free_dim_pad_per_rank=0,
            sbuf_byte_offset=0,
        )
    )

N_NODES = 50000
N_EDGES = 800000
D = 64
DP = 128  # padded row length (bf16) so each row is 256 B
C = 8
NPC = N_NODES // C  # 6250
P = 128
T = (NPC + P - 1) // P  # 49 dst tiles/core
LAST_ROWS = NPC - (T - 1) * P  # 106
SPLIT = 32768  # int16 index limit for dma_gather
BLKS_PER_CALL = 8  # indices per dma_gather call cap (1024 = ring limit)
NQ = 4  # SWDGE queues

_last_results = None


def _prep(edge_src, edge_dst, edge_weight):
    """Per-core layouts: wrapped int16 gather indices, bf16 one-hot S
    blocks, plus the uniform block structure (nbA/nbB per dst tile)."""
    order = np.argsort(edge_dst, kind="stable")
    src_s = edge_src[order].astype(np.int64)
    dst_s = edge_dst[order].astype(np.int64)
    w_s = edge_weight[order].astype(np.float32)

    cid = dst_s // NPC
    loc = dst_s - cid * NPC
    tid = loc // P
    half = (src_s >= SPLIT).astype(np.int64)  # 0=A, 1=B
    # sort key: (core, tile, half, src) — ascending src within each
    # (tile, half) segment gives the gather DMA monotone HBM addresses
    # (the one-hot S absorbs any edge order within a tile)
    key = ((cid * T + tid) * 2 + half) * N_NODES + src_s
    order2 = np.argsort(key, kind="stable")
    src_s, dst_s, w_s, cid, loc, tid, half = (
        a[order2] for a in (src_s, dst_s, w_s, cid, loc, tid, half)
    )
    key = ((cid * T + tid) * 2 + half)

    counts = np.bincount(key, minlength=C * T * 2).reshape(C, T, 2)
    nidxA = np.maximum(1, counts[:, :, 0].max(axis=0))  # [T] exact gather counts
    nidxB = counts[:, :, 1].max(axis=0)  # [T] may be 0
    nbA = (nidxA + P - 1) // P
    nbB = (nidxB + P - 1) // P
    nbt = nbA + nbB
    b0 = np.concatenate([[0], np.cumsum(nbt)])  # tile block offsets
    tb = int(b0[-1])

    # position of each edge inside its (core,tile,half) segment
    seg_starts = np.concatenate([[0], np.cumsum(counts.reshape(-1))])
    j = np.arange(len(dst_s)) - np.repeat(seg_starts[:-1], counts.reshape(-1))
    # block column (global within the [*, TB] block layout)
    half_off = np.where(half == 1, nbA[tid], 0)
    col = b0[tid] + half_off + j // P
    part = j % P

    # bf16 one-hot S blocks: S_all[e_pos, blk*128 + dstrel] = w
    s_all = np.zeros((C, P, tb * P), BF16)
    s_all[cid, part, col * P + (loc - tid * P)] = w_s.astype(BF16)

    # Decoupled gather streams: all A-blocks contiguous (tile order),
    # then all B-blocks — uniform 8-block gather calls span tile
    # boundaries.  Wrapped int16 index array [16, ncalls*64] replicated
    # to 128 partitions; call c of stream A reads idx cols [c*64, c*64+64).
    blkA_off = np.concatenate([[0], np.cumsum(nbA)])
    blkB_off = np.concatenate([[0], np.cumsum(nbB)])
    BA, BB = int(blkA_off[-1]), int(blkB_off[-1])
    nca = (BA + BLKS_PER_CALL - 1) // BLKS_PER_CALL
    ncb = (BB + BLKS_PER_CALL - 1) // BLKS_PER_CALL
    nblk_pad = (nca + ncb) * BLKS_PER_CALL
    bg = np.where(
        half == 1,
        nca * BLKS_PER_CALL + blkB_off[tid] + j // P,
        blkA_off[tid] + j // P,
    )
    pos = j % P
    idxw = np.zeros((C, 16, nblk_pad * 8), np.int16)
    idxw[cid, pos % 16, bg * 8 + pos // 16] = (src_s - half * SPLIT).astype(np.int16)

    nbA = [int(x) for x in nbA]
    nbB = [int(x) for x in nbB]
    nidxA = [int(x) for x in nidxA]
    nidxB = [int(x) for x in nidxB]
    b0 = [int(x) for x in b0]
    blkA_off = [int(x) for x in blkA_off]
    blkB_off = [int(x) for x in blkB_off]
    return s_all, idxw, nbA, nbB, nidxA, nidxB, blkA_off, blkB_off, nca, ncb, b0, tb


def _build(nbA, nbB, nidxA, nidxB, blkA_off, blkB_off, nca, ncb, b0, tb):
    nc = bacc.Bacc(num_swdge_queues=NQ, dynamic_dma_scratch_size=32768)
    f32 = mybir.dt.float32
    bf16 = mybir.dt.bfloat16
    i16 = mybir.dt.int16

    featb = nc.declare_dram_parameter("featb", [N_NODES, DP], bf16, isOutput=False)
    nblk_pad = (nca + ncb) * BLKS_PER_CALL
    idxw = nc.declare_dram_parameter("idxw", [P, nblk_pad * 8], i16, isOutput=False)
    s_all = nc.declare_dram_parameter("s_all", [P, tb * P], bf16, isOutput=False)
    featT = nc.declare_dram_parameter("featT", [D, T * P], bf16, isOutput=False)
    wt = nc.declare_dram_parameter("wt", [2 * D, D], bf16, isOutput=False)
    biasb = nc.declare_dram_parameter("biasb", [P, D], f32, isOutput=False)
    out = nc.declare_dram_parameter("out", [NPC, D], f32, isOutput=True)

    nbmax = max(a + b for a, b in zip(nbA, nbB))
    CB = BLKS_PER_CALL
    qrr = [0]  # round-robin SWDGE queue cursor

    def next_q():
        q = qrr[0]
        qrr[0] = (q + 1) % NQ
        return q

    with tile.TileContext(nc) as tc:
        with (
            tc.tile_pool(name="singles", bufs=1) as singles,
            tc.tile_pool(name="gapool", bufs=5) as gapool,
            tc.tile_pool(name="gbpool", bufs=5) as gbpool,
            tc.tile_pool(name="spool", bufs=6) as spool,
            tc.tile_pool(name="cpool", bufs=3) as cpool,
            tc.tile_pool(name="opool", bufs=3) as opool,
            tc.tile_pool(name="stat", bufs=6) as stat,
            tc.tile_pool(name="pnT", bufs=2, space="PSUM") as pnT,
            tc.tile_pool(name="pout", bufs=2, space="PSUM") as pout,
        ):
            idx_sb = singles.tile([P, nblk_pad * 8], i16)
            wt_sb = singles.tile([2 * D, D], bf16)
            bias_sb = singles.tile([P, D], f32)
            eps_sb = singles.tile([P, 1], f32)
            ha = blkA_off[min(4, T)] * 8
            hb0 = nca * BLKS_PER_CALL * 8
            hb = hb0 + blkB_off[min(4, T)] * 8
            nc.sync.dma_start(out=idx_sb[:, :ha], in_=idxw[:, :ha])
            nc.sync.dma_start(out=idx_sb[:, hb0:hb], in_=idxw[:, hb0:hb])
            nc.sync.dma_start(out=idx_sb[:, ha:hb0], in_=idxw[:, ha:hb0])
            nc.sync.dma_start(out=idx_sb[:, hb:], in_=idxw[:, hb:])
            nc.sync.dma_start(out=wt_sb[:], in_=wt[:])
            nc.sync.dma_start(out=bias_sb[:], in_=biasb[:])
            nc.vector.memset(eps_sb[:], 1e-24)

            # Decoupled gather streams: uniform 8-block calls spanning
            # tile boundaries (fewest calls under the 1024-idx ring cap).
            # Calls always gather full blocks — padded idx slots are 0 —
            # so every output slot is written (no stale-SBUF NaN hazard).
            ga = {}
            gb = {}
            issued = [0, 0]

            def issue_calls(stream, upto_block):
                store, base_tbl, coff, pool = (
                    (ga, 0, 0, gapool)
                    if stream == 0
                    else (gb, SPLIT, nca * CB, gbpool)
                )
                while issued[stream] * CB < upto_block:
                    c = issued[stream]
                    gt = pool.tile([P, CB * D], bf16, tag=f"g{stream}")
                    icol = (coff + c * CB) * 8
                    _dma_gather_half(
                        nc.gpsimd,
                        out_ap=gt[:].rearrange("p (n e) -> p n e", e=D),
                        in_ap=featb[base_tbl:, :],
                        idxs_ap=idx_sb[:, icol : icol + CB * 8],
                        num_idxs=CB * P,
                        elem_size=D,
                        queue_num=next_q(),
                    )
                    store[c] = gt
                    issued[stream] += 1

            for t in range(T):
                ka, kb = nbA[t], nbB[t]
                nbt = ka + kb
                issue_calls(0, blkA_off[t] + ka)
                issue_calls(1, blkB_off[t] + kb)
                s = spool.tile([P, nbmax * P], bf16, tag="s")
                nc.sync.dma_start(
                    out=s[:, : nbt * P],
                    in_=s_all[:, b0[t] * P : (b0[t] + nbt) * P],
                )
                nt = pnT.tile([D, P], f32)
                for i in range(nbt):
                    if i < ka:
                        blk = blkA_off[t] + i
                        gt = ga[blk // CB]
                    else:
                        blk = blkB_off[t] + (i - ka)
                        gt = gb[blk // CB]
                    off = blk % CB
                    nc.tensor.matmul(
                        out=nt[:],
                        lhsT=gt[:, off * D : (off + 1) * D],
                        rhs=s[:, i * P : (i + 1) * P],
                        start=(i == 0),
                        stop=(i == nbt - 1),
                    )
                comb = cpool.tile([P, P], bf16, tag="comb")
                nc.sync.dma_start(out=comb[:D, :], in_=featT[:, t * P : (t + 1) * P])
                nc.vector.tensor_copy(out=comb[D:, :], in_=nt[:])
                po = pout.tile([P, D], f32)
                nc.tensor.matmul(
                    out=po[:], lhsT=comb[:], rhs=wt_sb[:], start=True, stop=True
                )
                o = opool.tile([P, D], f32, tag="o")
                nc.vector.tensor_add(out=o[:], in0=po[:], in1=bias_sb[:])
                sq = opool.tile([P, D], f32, tag="sq")
                ssum = stat.tile([P, 1], f32, tag="ssum")
                nc.scalar.activation(
                    out=sq[:],
                    in_=o[:],
                    func=mybir.ActivationFunctionType.Square,
                    accum_out=ssum[:],
                )
                # sqrt(ssum + 1e-24) == max(||row||, 1e-12) up to rounding
                nrm = stat.tile([P, 1], f32, tag="nrm")
                nc.scalar.activation(
                    out=nrm[:],
                    in_=ssum[:],
                    func=mybir.ActivationFunctionType.Sqrt,
                    bias=eps_sb[:],
                )
                rin = stat.tile([P, 1], f32, tag="rin")
                nc.vector.reciprocal(out=rin[:], in_=nrm[:])
                on = opool.tile([P, D], f32, tag="on")
                nc.scalar.activation(
                    out=on[:],
                    in_=o[:],
                    func=mybir.ActivationFunctionType.Copy,
                    scale=rin[:],
                )
                rows = LAST_ROWS if t == T - 1 else P
                nc.sync.dma_start(out=out[t * P : t * P + rows, :], in_=on[:rows, :])

    nc.compile()
    return nc


def kernel(features, edge_src, edge_dst, edge_weight, W, b, _cache={}):
    global _last_results
    features = np.ascontiguousarray(features, dtype=np.float32)
    edge_src = np.ascontiguousarray(edge_src, dtype=np.int32)
    edge_dst = np.ascontiguousarray(edge_dst, dtype=np.int32)
    edge_weight = np.ascontiguousarray(edge_weight, dtype=np.float32)
    W = np.ascontiguousarray(W, dtype=np.float32)
    b = np.ascontiguousarray(b, dtype=np.float32)

    s_all, idxw, nbA, nbB, nidxA, nidxB, blkA_off, blkB_off, nca, ncb, b0, tb = _prep(
        edge_src, edge_dst, edge_weight
    )

    featb = np.zeros((N_NODES, DP), BF16)
    featb[:, :D] = features.astype(BF16)
    featT = features.T.astype(BF16)
    featT_pad = np.zeros((C, D, T * P), BF16)
    for c in range(C):
        featT_pad[c, :, :NPC] = featT[:, c * NPC : (c + 1) * NPC]
    wt = np.ascontiguousarray(W.T).astype(BF16)
    biasb = np.ascontiguousarray(np.broadcast_to(b, (P, D))).astype(np.float32)

    key = ("k9", tb, tuple(nidxA), tuple(nidxB))
    if key not in _cache:
        _cache.clear()
        _cache[key] = _build(nbA, nbB, nidxA, nidxB, blkA_off, blkB_off, nca, ncb, b0, tb)
    nc = _cache[key]

    in_maps = [
        {
            "featb": featb,
            "idxw": np.ascontiguousarray(np.tile(idxw[c], (8, 1))),
            "s_all": np.ascontiguousarray(s_all[c]),
            "featT": featT_pad[c],
            "wt": wt,
            "biasb": biasb,
        }
        for c in range(C)
    ]
    import os

    trace = bool(os.environ.get("GS_TRACE"))
    if trace:
        try:
            import antenv.axon_hooks  # noqa: F401  (profiling-only dep)
        except ImportError:
            trace = False
    res = run_bass_kernel_spmd(
        nc, in_maps, core_ids=list(range(C)), trace=trace
    )
    _last_results = res
    out = np.concatenate([res.results[c]["out"] for c in range(C)], axis=0)
    return out.astype(np.float32)


# revision 35
# speedup vs baseline: 1.2452x; 1.2452x over previous
"""GraphSAGE layer on 8 Trainium2 NeuronCores.  ~287 us HW exec
(baseline 977 us).

Strategy (1D graph partitioning):
  - Nodes (output rows / destination segments) sharded across 8 cores,
    6250 per core.  Edges are bucketed to the core owning their
    destination, sorted by (dst tile, src); the full feature table is
    replicated in DRAM on every core as bf16 rows padded to 128 cols
    (so the row stride is 256 B, required by the gather descriptor
    stride encoding).
  - Source rows are fetched with `dma_gather` (SWDGE gather, one
    descriptor per edge), round-robined over 4 SWDGE queues so all four
    Q7 core-pairs generate descriptors in parallel (~4x the single-queue
    rate; this is the critical path).  Each call is <= 1024 indices
    (the per-queue descriptor-ring cap), split evenly per (tile, half).
    Only the 128 B of real bf16 features per row are gathered (raw
    instruction builder `_dma_gather_half` skips the 256 B elem assert,
    which only the transpose path needs).  Sorting edges by src gives
    the drain monotone HBM addresses.  Indices are int16, so the table
    is split at row 32768 (A/B halves).
  - The one-hot segment matrices S ([128 edge-pos x 128 dst] * weight)
    are precomputed on the host in bf16 and streamed from DRAM per
    destination tile — no on-chip one-hot construction (the DVE
    per-partition-scalar path costs ~1.5-3 us per block).
  - Per 128-destination tile the kernel accumulates
        neighborT[64, 128] += G_block.T @ S_block
    in PSUM with single-pass bf16 matmuls (~107 ns effective each).
  - Self features arrive pre-transposed bf16 from the host; the final
    linear layer is one [128,128]x[128,64] bf16 matmul per tile, then
    bias add (DVE), sqrt(sum(x^2)+1e-24) (ACT, == max(||x||,1e-12)) and
    row scaling by 1/norm (ACT Copy with per-partition scale) in f32.
"""

import sys

if "/opt/trn_rl_repo" not in sys.path:
    sys.path.insert(0, "/opt/trn_rl_repo")

import numpy as np
import ml_dtypes

import concourse.bacc as bacc
import concourse.tile as tile
from concourse import mybir
from concourse.bass_utils import run_bass_kernel_spmd

BF16 = ml_dtypes.bfloat16


def _dma_gather_half(gp, out_ap, in_ap, idxs_ap, num_idxs, elem_size, queue_num):
    """dma_gather (non-transpose, DRAM source) with elem_size_bytes that is
    not a multiple of 256 B.  Mirrors BassGpSimd.dma_gather minus the
    256 B-elem assert — that alignment is only required by the transpose
    path; the non-transpose ucode emits one descriptor of
    elem_size_bytes per index with a row stride that must be 256 B
    aligned (stride_bytes_256 encoding)."""
    from concourse import ap_utils
    from concourse.bass import exact_div, round_up_to_multiple

    assert idxs_ap.dtype == mybir.dt.int16
    assert in_ap.dtype == out_ap.dtype
    elem_step = in_ap.ap[0][0]
    stride_bytes = elem_step * mybir.dt.size(in_ap.dtype)
    stride_bytes_256 = exact_div(stride_bytes, 256)
    assert ap_utils.ap_is_contiguous(out_ap.ap[1:])
    assert ap_utils.ap_is_contiguous(idxs_ap.ap[1:])
    assert out_ap.ap[0][1] * out_ap.ap[1][1] == round_up_to_multiple(num_idxs, 128)
    assert out_ap.ap[-1][1] == elem_size

    _in_ap = gp.lower_ap_dma(in_ap, for_custom_bir_dma=True)
    _idxs_ap = gp.lower_ap(idxs_ap)
    _out_ap = gp.lower_ap(out_ap)
    return gp.add_instruction(
        mybir.InstDMAGatherAnt(
            name=gp.bass.get_next_instruction_name(),
            ins=[
                *_in_ap,
                _idxs_ap,
                gp.lower_val_access(gp.to_reg(num_idxs)),
            ],
            outs=[_out_ap],
            transpose=False,
            num_idxs=num_idxs,
            elem_size=elem_size,
            stride_bytes_256=stride_bytes_256,
            gen_mode=0,
            single_packet=True,
            queue_num=queue_num,
            sbuf_tokens_per_rank=0,
            sbuf_free_dim_per_rank=0,
            sbuf_free_dim_pad_per_rank=0,
            sbuf_byte_offset=0,
        )
    )

N_NODES = 50000
N_EDGES = 800000
D = 64
DP = 128  # padded row length (bf16) so each row is 256 B
C = 8
NPC = N_NODES // C  # 6250
P = 128
T = (NPC + P - 1) // P  # 49 dst tiles/core
LAST_ROWS = NPC - (T - 1) * P  # 106
SPLIT = 32768  # int16 index limit for dma_gather
BLKS_PER_CALL = 8  # indices per dma_gather call cap (1024 = ring limit)
NQ = 4  # SWDGE queues

_last_results = None


def _prep(edge_src, edge_dst, edge_weight):
    """Per-core layouts: wrapped int16 gather indices, bf16 one-hot S
    blocks, plus the uniform block structure (nbA/nbB per dst tile)."""
    order = np.argsort(edge_dst, kind="stable")
    src_s = edge_src[order].astype(np.int64)
    dst_s = edge_dst[order].astype(np.int64)
    w_s = edge_weight[order].astype(np.float32)

    cid = dst_s // NPC
    loc = dst_s - cid * NPC
    tid = loc // P
    half = (src_s >= SPLIT).astype(np.int64)  # 0=A, 1=B
    # sort key: (core, tile, half, src) — ascending src within each
    # (tile, half) segment gives the gather DMA monotone HBM addresses
    # (the one-hot S absorbs any edge order within a tile)
    key = ((cid * T + tid) * 2 + half) * N_NODES + src_s
    order2 = np.argsort(key, kind="stable")
    src_s, dst_s, w_s, cid, loc, tid, half = (
        a[order2] for a in (src_s, dst_s, w_s, cid, loc, tid, half)
    )
    key = ((cid * T + tid) * 2 + half)

    counts = np.bincount(key, minlength=C * T * 2).reshape(C, T, 2)
    nidxA = np.maximum(1, counts[:, :, 0].max(axis=0))  # [T] exact gather counts
    nidxB = counts[:, :, 1].max(axis=0)  # [T] may be 0
    nbA = (nidxA + P - 1) // P
    nbB = (nidxB + P - 1) // P
    nbt = nbA + nbB
    b0 = np.concatenate([[0], np.cumsum(nbt)])  # tile block offsets
    tb = int(b0[-1])

    # position of each edge inside its (core,tile,half) segment
    seg_starts = np.concatenate([[0], np.cumsum(counts.reshape(-1))])
    j = np.arange(len(dst_s)) - np.repeat(seg_starts[:-1], counts.reshape(-1))
    # block column (global within the [*, TB] block layout)
    half_off = np.where(half == 1, nbA[tid], 0)
    col = b0[tid] + half_off + j // P
    part = j % P

    # bf16 one-hot S blocks: S_all[e_pos, blk*128 + dstrel] = w
    s_all = np.zeros((C, P, tb * P), BF16)
    s_all[cid, part, col * P + (loc - tid * P)] = w_s.astype(BF16)

    # Decoupled gather streams: all A-blocks contiguous (tile order),
    # then all B-blocks — uniform 8-block gather calls span tile
    # boundaries.  Wrapped int16 index array [16, ncalls*64] replicated
    # to 128 partitions; call c of stream A reads idx cols [c*64, c*64+64).
    blkA_off = np.concatenate([[0], np.cumsum(nbA)])
    blkB_off = np.concatenate([[0], np.cumsum(nbB)])
    BA, BB = int(blkA_off[-1]), int(blkB_off[-1])
    nca = (BA + BLKS_PER_CALL - 1) // BLKS_PER_CALL
    ncb = (BB + BLKS_PER_CALL - 1) // BLKS_PER_CALL
    nblk_pad = (nca + ncb) * BLKS_PER_CALL
    bg = np.where(
        half == 1,
        nca * BLKS_PER_CALL + blkB_off[tid] + j // P,
        blkA_off[tid] + j // P,
    )
    pos = j % P
    idxw = np.zeros((C, 16, nblk_pad * 8), np.int16)
    idxw[cid, pos % 16, bg * 8 + pos // 16] = (src_s - half * SPLIT).astype(np.int16)

    nbA = [int(x) for x in nbA]
    nbB = [int(x) for x in nbB]
    nidxA = [int(x) for x in nidxA]
    nidxB = [int(x) for x in nidxB]
    b0 = [int(x) for x in b0]
    blkA_off = [int(x) for x in blkA_off]
    blkB_off = [int(x) for x in blkB_off]
    return s_all, idxw, nbA, nbB, nidxA, nidxB, blkA_off, blkB_off, nca, ncb, b0, tb


def _build(nbA, nbB, nidxA, nidxB, blkA_off, blkB_off, nca, ncb, b0, tb):
    nc = bacc.Bacc(num_swdge_queues=NQ, dynamic_dma_scratch_size=32768)
    f32 = mybir.dt.float32
    bf16 = mybir.dt.bfloat16
    i16 = mybir.dt.int16

    featb = nc.declare_dram_parameter("featb", [N_NODES, DP], bf16, isOutput=False)
    nblk_pad = (nca + ncb) * BLKS_PER_CALL
    idxw = nc.declare_dram_parameter("idxw", [P, nblk_pad * 8], i16, isOutput=False)
    s_all = nc.declare_dram_parameter("s_all", [P, tb * P], bf16, isOutput=False)
    featT = nc.declare_dram_parameter("featT", [D, T * P], bf16, isOutput=False)
    wt = nc.declare_dram_parameter("wt", [2 * D, D], bf16, isOutput=False)
    biasb = nc.declare_dram_parameter("biasb", [P, D], f32, isOutput=False)
    out = nc.declare_dram_parameter("out", [NPC, D], f32, isOutput=True)

    nbmax = max(a + b for a, b in zip(nbA, nbB))
    CB = BLKS_PER_CALL
    qrr = [0]  # round-robin SWDGE queue cursor

    def next_q():
        q = qrr[0]
        qrr[0] = (q + 1) % NQ
        return q

    with tile.TileContext(nc) as tc:
        with (
            tc.tile_pool(name="singles", bufs=1) as singles,
            tc.tile_pool(name="gpool", bufs=5) as gpool,
            tc.tile_pool(name="spool", bufs=6) as spool,
            tc.tile_pool(name="cpool", bufs=3) as cpool,
            tc.tile_pool(name="opool", bufs=3) as opool,
            tc.tile_pool(name="stat", bufs=6) as stat,
            tc.tile_pool(name="pnT", bufs=2, space="PSUM") as pnT,
            tc.tile_pool(name="pout", bufs=2, space="PSUM") as pout,
        ):
            idx_sb = singles.tile([P, nblk_pad * 8], i16)
            wt_sb = singles.tile([2 * D, D], bf16)
            bias_sb = singles.tile([P, D], f32)
            eps_sb = singles.tile([P, 1], f32)
            ha = blkA_off[min(4, T)] * 8
            hb0 = nca * BLKS_PER_CALL * 8
            hb = hb0 + blkB_off[min(4, T)] * 8
            nc.sync.dma_start(out=idx_sb[:, :ha], in_=idxw[:, :ha])
            nc.sync.dma_start(out=idx_sb[:, hb0:hb], in_=idxw[:, hb0:hb])
            nc.sync.dma_start(out=idx_sb[:, ha:hb0], in_=idxw[:, ha:hb0])
            nc.sync.dma_start(out=idx_sb[:, hb:], in_=idxw[:, hb:])
            nc.sync.dma_start(out=wt_sb[:], in_=wt[:])
            nc.sync.dma_start(out=bias_sb[:], in_=biasb[:])
            nc.vector.memset(eps_sb[:], 1e-24)

            wm = [0] * 5  # per-gpool-buffer written extent (blocks)
            for tpos, t in enumerate(range(T)):
                ka, kb = nbA[t], nbB[t]
                nbt = ka + kb
                # exact index counts leave the tail of the last block per
                # half stale; that is only safe where this pool buffer was
                # already written by an earlier tile (finite bf16, w=0 in S)
                safe_exact = nbt <= wm[tpos % 5]
                wm[tpos % 5] = max(wm[tpos % 5], nbt)
                g = gpool.tile([P, nbmax * D], bf16, tag="g")
                # per-(tile,half) gather calls, split into equal sizes of
                # <= BLKS_PER_CALL blocks; padded index slots are 0
                # (gather row 0) so no stale SBUF data reaches the matmul
                for base_tbl, nblk, nidx_half, coff, boff in (
                    (0, ka, nidxA[t], 0, blkA_off[t]),
                    (SPLIT, kb, nidxB[t], ka, nca * CB + blkB_off[t]),
                ):
                    ncalls = (nblk + CB - 1) // CB
                    splits = [
                        nblk * i // ncalls for i in range(ncalls + 1)
                    ] if ncalls else [0]
                    for k0, k1 in zip(splits[:-1], splits[1:]):
                        if safe_exact:
                            nidx = min(k1 * P, nidx_half) - k0 * P
                        else:
                            nidx = (k1 - k0) * P
                        _dma_gather_half(
                            nc.gpsimd,
                            out_ap=g[
                                :, (coff + k0) * D : (coff + k1) * D
                            ].rearrange("p (n e) -> p n e", e=D),
                            in_ap=featb[base_tbl:, :],
                            idxs_ap=idx_sb[
                                :, (boff + k0) * 8 : (boff + k0) * 8 + (nidx + 15) // 16
                            ],
                            num_idxs=nidx,
                            elem_size=D,
                            queue_num=next_q(),
                        )
                s = spool.tile([P, nbmax * P], bf16, tag="s")
                nc.sync.dma_start(
                    out=s[:, : nbt * P],
                    in_=s_all[:, b0[t] * P : (b0[t] + nbt) * P],
                )
                nt = pnT.tile([D, P], f32)
                for i in range(nbt):
                    nc.tensor.matmul(
                        out=nt[:],
                        lhsT=g[:, i * D : (i + 1) * D],
                        rhs=s[:, i * P : (i + 1) * P],
                        start=(i == 0),
                        stop=(i == nbt - 1),
                    )
                comb = cpool.tile([P, P], bf16, tag="comb")
                nc.sync.dma_start(out=comb[:D, :], in_=featT[:, t * P : (t + 1) * P])
                nc.vector.tensor_copy(out=comb[D:, :], in_=nt[:])
                po = pout.tile([P, D], f32)
                nc.tensor.matmul(
                    out=po[:], lhsT=comb[:], rhs=wt_sb[:], start=True, stop=True
                )
                o = opool.tile([P, D], f32, tag="o")
                nc.vector.tensor_add(out=o[:], in0=po[:], in1=bias_sb[:])
                sq = opool.tile([P, D], f32, tag="sq")
                ssum = stat.tile([P, 1], f32, tag="ssum")
                nc.scalar.activation(
                    out=sq[:],
                    in_=o[:],
                    func=mybir.ActivationFunctionType.Square,
                    accum_out=ssum[:],
                )
                # sqrt(ssum + 1e-24) == max(||row||, 1e-12) up to rounding
                nrm = stat.tile([P, 1], f32, tag="nrm")
                nc.scalar.activation(
                    out=nrm[:],
                    in_=ssum[:],
                    func=mybir.ActivationFunctionType.Sqrt,
                    bias=eps_sb[:],
                )
                rin = stat.tile([P, 1], f32, tag="rin")
                nc.vector.reciprocal(out=rin[:], in_=nrm[:])
                on = opool.tile([P, D], f32, tag="on")
                nc.scalar.activation(
                    out=on[:],
                    in_=o[:],
                    func=mybir.ActivationFunctionType.Copy,
                    scale=rin[:],
                )
                rows = LAST_ROWS if t == T - 1 else P
                nc.sync.dma_start(out=out[t * P : t * P + rows, :], in_=on[:rows, :])

    nc.compile()
    return nc


def kernel(features, edge_src, edge_dst, edge_weight, W, b, _cache={}):
    global _last_results
    features = np.ascontiguousarray(features, dtype=np.float32)
    edge_src = np.ascontiguousarray(edge_src, dtype=np.int32)
    edge_dst = np.ascontiguousarray(edge_dst, dtype=np.int32)
    edge_weight = np.ascontiguousarray(edge_weight, dtype=np.float32)
    W = np.ascontiguousarray(W, dtype=np.float32)
    b = np.ascontiguousarray(b, dtype=np.float32)

    s_all, idxw, nbA, nbB, nidxA, nidxB, blkA_off, blkB_off, nca, ncb, b0, tb = _prep(
        edge_src, edge_dst, edge_weight
    )

    featb = np.zeros((N_NODES, DP), BF16)
    featb[:, :D] = features.astype(BF16)
    featT = features.T.astype(BF16)
    featT_pad = np.zeros((C, D, T * P), BF16)
    for c in range(C):
        featT_pad[c, :, :NPC] = featT[:, c * NPC : (c + 1) * NPC]
    wt = np.ascontiguousarray(W.T).astype(BF16)
    biasb = np.ascontiguousarray(np.broadcast_to(b, (P, D))).astype(np.float32)

    key = ("k8", tb, tuple(nidxA), tuple(nidxB))
    if key not in _cache:
        _cache.clear()
        _cache[key] = _build(nbA, nbB, nidxA, nidxB, blkA_off, blkB_off, nca, ncb, b0, tb)
    nc = _cache[key]

    in_maps = [
        {
            "featb": featb,
            "idxw": np.ascontiguousarray(np.tile(idxw[c], (8, 1))),
            "s_all": np.ascontiguousarray(s_all[c]),
            "featT": featT_pad[c],
            "wt": wt,
            "biasb": biasb,
        }
        for c in range(C)
    ]
    import os

    trace = bool(os.environ.get("GS_TRACE"))
    if trace:
        try:
            import antenv.axon_hooks  # noqa: F401  (profiling-only dep)
        except ImportError:
            trace = False
    res = run_bass_kernel_spmd(
        nc, in_maps, core_ids=list(range(C)), trace=trace
    )
    _last_results = res
    out = np.concatenate([res.results[c]["out"] for c in range(C)], axis=0)
    return out.astype(np.float32)
